# revision 6
# baseline (speedup 1.0000x reference)
"""Trainium2 Bass kernel for nn_AggregationEncoder (gnn_message_passing).

Reference computation:
    adj[g, m] = 1 where an edge (g, m) exists (set semantics)
    norm[m]   = max(sum_g adj[g, m], 1)
    out[b, m, d] = sum_g adj[g, m] / norm[m] * x[b, g, d]

Structural facts hardcoded from the problem spec:
  - x: [B=2, G=40962, D=512] float32
  - edge_index: [E=122880, 2] int64, BOTH columns in [0, 2562), so the
    contraction only involves x[:, :2562, :].
  - M = 2562 mesh nodes.

Sharding (8 cores): 2 batches x 4 mesh-column chunks of W=642 columns.
Host work is sharding/layout only: dedup the edge set, lay it out as a
dense 0/1 fp8e4m3 adjacency chunk (partition-major [128, 21, 642]) that
the device DMAs directly, pre-cast x to bf16 in the same partition-major
layout, and precompute per-column reciprocal degrees (a pure function of
edge_index).

Ragged-dimension trick (2562 = 4*642 - 6, 642 = 5*128 + 2): columns
0..639 come from the main 21-ktile matmul stream over senders 0..2559
(+ leftover senders 2560/2561 whose edges live in k-tile 20 rows 0/1).
Columns 640/641 come from ONE extra matmul contracting only k-tile 20:
the host gathers those receivers' sender rows into x pad rows
2562..2687 and marks them in adjacency columns 640/641.

Device-side (per core): big partition-major DMAs (x on SP ring, adj on
ACT ring, small first chunks so the PE starts early); 21x5+1
accumulating matmuls into 6 PSUM banks; per-partition reciprocal scale
on DVE/ScalarE interleaved with the final k-group so output drains
overlap the stream tail; output DMAs alternate rings.
"""

import numpy as np
import ml_dtypes

B = 2
G = 40962
D = 512
M = 2562            # mesh nodes
SEN = 2562          # senders (edge values < 2562)
GP = 2688           # padded sender rows = 21*128
KT = GP // 128      # 21 k-tiles
NQ = 4              # mesh-column chunks
W = 642             # mesh columns per chunk (5*128 + 2)
WMAIN = 640         # columns via the main 21-ktile stream
NMT = 5             # full 128-col m-tiles
PAD0 = 2562         # first gather-pad row
NPAD = GP - PAD0    # 126 gather slots
N_CORES = 8

SC = 7              # input DMA chunks of PL=3 k-tiles
PL = 3

_NC_CACHE = None


def _build_bass():
    import concourse.bacc as bacc
    import concourse.mybir as mybir
    import concourse.tile as tile

    dt = mybir.dt
    nc = bacc.Bacc("TRN2", target_bir_lowering=False, debug=False,
                   num_devices=N_CORES)

    # DRAM layout: 3-k-plane blocks, partition-minor inside each block —
    # every DMA descriptor is one partition's 3-plane run (3KB x / 1.9KB
    # adj) and consecutive descriptors are sequential in DRAM (fast HBM
    # streaming; strided descriptor streams measured ~3x slower).
    xh = nc.dram_tensor("xh", [SC, 128, PL, D], dt.bfloat16,
                        kind="ExternalInput")
    adj = nc.dram_tensor("adj", [SC, 128, PL, W], dt.float8e4,
                         kind="ExternalInput")
    recip = nc.dram_tensor("recip", [128, 6], dt.float32,
                           kind="ExternalInput")
    out = nc.dram_tensor("out", [W, D], dt.float32, kind="ExternalOutput")

    with tile.TileContext(nc) as tc:
        with (
            tc.tile_pool(name="sbuf", bufs=1) as sb,
            tc.tile_pool(name="outb", bufs=2) as outb,
            tc.tile_pool(name="psum", bufs=1, space="PSUM") as ps,
        ):
            a_sb = sb.tile([128, KT, W], dt.float8e4)
            x_sb = sb.tile([128, KT, D], dt.bfloat16)
            recip_sb = sb.tile([128, 6], dt.float32)
            for s in range(SC):
                nc.scalar.dma_start(out=a_sb[:, s * PL:(s + 1) * PL, :],
                                    in_=adj[s])
                nc.sync.dma_start(out=x_sb[:, s * PL:(s + 1) * PL, :],
                                  in_=xh[s])
            # recip is only needed at drain time; keep it behind the adj
            # chunks so it doesn't delay the first matmul.
            nc.scalar.dma_start(recip_sb[:], recip[:])

            psums = [ps.tile([128, D], dt.float32, tag=f"ps{mt}",
                             name=f"psum{mt}")
                     for mt in range(NMT)]
            pst = ps.tile([2, D], dt.float32, tag="pst", name="psum_tiny")

            def mm(mt, kt):
                nc.tensor.matmul(
                    psums[mt][:, :],
                    lhsT=a_sb[:, kt, mt * 128:(mt + 1) * 128],
                    rhs=x_sb[:, kt, :],
                    start=(kt == 0),
                    stop=(kt == KT - 1),
                )

            for t in range(6):               # kts 0..17
                for mt in range(NMT):
                    for kt in (3 * t, 3 * t + 1, 3 * t + 2):
                        mm(mt, kt)

            # Final k-group: finish each m-tile then immediately drain it
            # (normalize on alternating engines, store on alternating
            # rings) so only the last tile's drain is exposed.
            def drain(mt):
                o_sb = outb.tile([128, D], dt.float32, tag=f"osb{mt % 2}",
                                 name=f"osb{mt}")
                if mt % 2 == 0:
                    nc.vector.tensor_scalar_mul(
                        o_sb[:], psums[mt][:, 0:D], recip_sb[:, mt:mt + 1])
                    nc.sync.dma_start(out[mt * 128:(mt + 1) * 128, :],
                                      o_sb[:])
                else:
                    nc.scalar.activation(
                        o_sb[:], psums[mt][:, 0:D],
                        mybir.ActivationFunctionType.Copy,
                        scale=recip_sb[:, mt:mt + 1])
                    nc.scalar.dma_start(out[mt * 128:(mt + 1) * 128, :],
                                        o_sb[:])

            for mt in range(NMT):
                for kt in (18, 19, 20):
                    mm(mt, kt)
                if mt == NMT - 1:
                    # tiny columns 640/641: contract only k-tile 20
                    # (gathered rows + leftover senders 2560/2561)
                    nc.tensor.matmul(
                        pst[:, :],
                        lhsT=a_sb[:, KT - 1, WMAIN:W],
                        rhs=x_sb[:, KT - 1, :],
                        start=True,
                        stop=True,
                    )
                drain(mt)

            ot = outb.tile([2, D], dt.float32, tag="osbt", name="osb_t")
            nc.scalar.activation(ot[:], pst[:, 0:D],
                                 mybir.ActivationFunctionType.Copy,
                                 scale=recip_sb[0:2, 5:6])
            nc.scalar.dma_start(out[WMAIN:W, :], ot[:])

    nc.finalize()
    return nc


def _get_nc():
    global _NC_CACHE
    if _NC_CACHE is None:
        _NC_CACHE = _build_bass()
    return _NC_CACHE


def _pm(a):
    """[GP, F] row-major -> [SC, 128, PL, F] 3-plane partition-minor."""
    return np.ascontiguousarray(
        a.reshape(SC, PL, 128, a.shape[1]).transpose(0, 2, 1, 3))


def _host_shard(grid_node_features, edge_index):
    """Dedup edges and lay them out as per-chunk dense adjacency + padded
    bf16 x + reciprocal degrees. Returns per-core input maps."""
    x = np.asarray(grid_node_features)
    e = np.asarray(edge_index)
    g = e[:, 0].astype(np.int64)
    m = e[:, 1].astype(np.int64)
    key = np.unique(g * M + m)           # set semantics
    g = key // M
    m = key % M
    deg = np.bincount(m, minlength=M).astype(np.float64)
    rec_full = (1.0 / np.maximum(deg, 1.0)).astype(np.float32)

    ONE8 = np.uint8(0x38)                # fp8 e4m3 1.0

    adjs = []
    recs = []
    glists = []
    for q in range(NQ):
        lo = q * W
        sel = (m >= lo) & (m < lo + W)
        gq = g[sel]
        mq = m[sel] - lo
        av = np.zeros((GP, W), np.uint8)
        # main columns: all senders (incl. 2560/2561 -> k-tile 20 rows)
        main = mq < WMAIN
        av[gq[main], mq[main]] = ONE8
        # tiny columns 640/641: senders >= 2560 sit in k-tile 20 already;
        # senders < 2560 are gathered into pad rows.
        glist = []
        for r in (WMAIN, WMAIN + 1):
            if lo + r >= M:
                continue
            snd = np.sort(gq[mq == r])
            av[snd[snd >= 2560], r] = ONE8
            for s in snd[snd < 2560]:
                av[PAD0 + len(glist), r] = ONE8
                glist.append(s)
        if len(glist) > NPAD:
            raise ValueError(f"gather overflow: {len(glist)} > {NPAD}")
        adjs.append(_pm(av).view(ml_dtypes.float8_e4m3))
        glists.append(np.asarray(glist, np.int64))

        rv = np.zeros((128, 6), np.float32)
        for mt in range(NMT):
            c0 = lo + mt * 128
            n = min(128, max(M - c0, 0))
            if n > 0:
                rv[:n, mt] = rec_full[c0:c0 + n]
        for j in range(2):
            if lo + WMAIN + j < M:
                rv[j, 5] = rec_full[lo + WMAIN + j]
        recs.append(rv)

    in_maps = [None] * N_CORES
    for b in range(B):
        xb = x[b, :SEN, :].astype(ml_dtypes.bfloat16)
        for q in range(NQ):
            xp = np.zeros((GP, D), ml_dtypes.bfloat16)
            xp[:SEN] = xb
            gl = glists[q]
            if gl.size:
                xp[PAD0:PAD0 + gl.size] = xb[gl]
            in_maps[b * NQ + q] = {"xh": _pm(xp), "adj": adjs[q],
                                   "recip": recs[q]}
    return in_maps


def kernel(grid_node_features, edge_index):
    from concourse.bass_utils import run_bass_kernel_spmd

    nc = _get_nc()
    in_maps = _host_shard(grid_node_features, edge_index)
    res = run_bass_kernel_spmd(nc, in_maps, core_ids=list(range(N_CORES)))

    out = np.empty((B, M, D), dtype=np.float32)
    for c in range(N_CORES):
        b, q = divmod(c, NQ)
        lo = q * W
        cq = min(W, M - lo)
        out[b, lo:lo + cq, :] = res.results[c]["out"][:cq, :]
    return out


# revision 8
# speedup vs baseline: 1.1381x; 1.1381x over previous
"""Trainium2 Bass kernel for nn_AggregationEncoder (gnn_message_passing).

Reference computation:
    adj[g, m] = 1 where an edge (g, m) exists (set semantics)
    norm[m]   = max(sum_g adj[g, m], 1)
    out[b, m, d] = sum_g adj[g, m] / norm[m] * x[b, g, d]

Structural facts hardcoded from the problem spec:
  - x: [B=2, G=40962, D=512] float32
  - edge_index: [E=122880, 2] int64, BOTH columns in [0, 2562), so the
    contraction only involves x[:, :2562, :].
  - M = 2562 mesh nodes.

Sharding (8 cores): 2 batches x 4 mesh-column chunks of W=642 columns.
Host work is sharding/layout only: dedup the edge set, lay it out as a
dense 0/1 bf16 adjacency chunk (partition-major [128, 21, 642]) that
the device DMAs directly, pre-cast x to bf16 in the same partition-major
layout, and precompute per-column reciprocal degrees (a pure function of
edge_index).

Ragged-dimension trick (2562 = 4*642 - 6, 642 = 5*128 + 2): columns
0..639 come from the main 21-ktile matmul stream over senders 0..2559
(+ leftover senders 2560/2561 whose edges live in k-tile 20 rows 0/1).
Columns 640/641 come from ONE extra matmul contracting only k-tile 20:
the host gathers those receivers' sender rows into x pad rows
2562..2687 and marks them in adjacency columns 640/641.

Device-side (per core): big partition-major DMAs (x on SP ring, adj on
ACT ring, small first chunks so the PE starts early); 21x5+1
accumulating matmuls into 6 PSUM banks; per-partition reciprocal scale
on DVE/ScalarE interleaved with the final k-group so output drains
overlap the stream tail; output DMAs alternate rings.
"""

import numpy as np
import ml_dtypes

B = 2
G = 40962
D = 512
M = 2562            # mesh nodes
SEN = 2562          # senders (edge values < 2562)
GP = 2688           # padded sender rows = 21*128
KT = GP // 128      # 21 k-tiles
NQ = 4              # mesh-column chunks
W = 642             # mesh columns per chunk (5*128 + 2)
WMAIN = 640         # columns via the main 21-ktile stream
NMT = 5             # full 128-col m-tiles
PAD0 = 2562         # first gather-pad row
NPAD = GP - PAD0    # 126 gather slots
N_CORES = 8

SC = 7              # input DMA chunks of PL=3 k-tiles
PL = 3

_NC_CACHE = None


def _build_bass():
    import concourse.bacc as bacc
    import concourse.mybir as mybir
    import concourse.tile as tile

    dt = mybir.dt
    nc = bacc.Bacc("TRN2", target_bir_lowering=False, debug=False,
                   num_devices=N_CORES)

    # DRAM layout: 3-k-plane blocks, partition-minor inside each block —
    # every DMA descriptor is one partition's 3-plane run (3KB x / 1.9KB
    # adj) and consecutive descriptors are sequential in DRAM (fast HBM
    # streaming; strided descriptor streams measured ~3x slower).
    xh = nc.dram_tensor("xh", [SC, 128, PL, D], dt.bfloat16,
                        kind="ExternalInput")
    adj = nc.dram_tensor("adj", [SC, 128, PL, W], dt.bfloat16,
                         kind="ExternalInput")
    recip = nc.dram_tensor("recip", [128, 6], dt.float32,
                           kind="ExternalInput")
    out = nc.dram_tensor("out", [W, D], dt.float32, kind="ExternalOutput")

    with tile.TileContext(nc) as tc:
        with (
            tc.tile_pool(name="sbuf", bufs=1) as sb,
            tc.tile_pool(name="outb", bufs=2) as outb,
            tc.tile_pool(name="psum", bufs=1, space="PSUM") as ps,
        ):
            a_sb = sb.tile([128, KT, W], dt.bfloat16)
            x_sb = sb.tile([128, KT, D], dt.bfloat16)
            recip_sb = sb.tile([128, 6], dt.float32)
            for s in range(SC):
                nc.scalar.dma_start(out=a_sb[:, s * PL:(s + 1) * PL, :],
                                    in_=adj[s])
                nc.sync.dma_start(out=x_sb[:, s * PL:(s + 1) * PL, :],
                                  in_=xh[s])
            # recip is only needed at drain time; keep it behind the adj
            # chunks so it doesn't delay the first matmul.
            nc.scalar.dma_start(recip_sb[:], recip[:])

            psums = [ps.tile([128, D], dt.float32, tag=f"ps{mt}",
                             name=f"psum{mt}")
                     for mt in range(NMT)]
            pst = ps.tile([2, D], dt.float32, tag="pst", name="psum_tiny")

            # Warm-up matmuls: PE clock needs ~3us of continuous work to
            # reach 2.4GHz; fill the pre-data window (~7.3-10.5us, DMA in
            # flight) so the real stream starts at speed. Depends only on
            # memsets, so these issue right after the boot preamble.
            warm_src = sb.tile([128, D], dt.bfloat16)
            nc.vector.memset(warm_src[:], 1.0)
            warm = ps.tile([32, D], dt.float32, tag="warm", name="warm")
            for _ in range(4):
                nc.tensor.matmul(warm[:, :], lhsT=warm_src[:, 0:32],
                                 rhs=warm_src[:], start=True, stop=True)

            def mm(mt, kt):
                nc.tensor.matmul(
                    psums[mt][:, :],
                    lhsT=a_sb[:, kt, mt * 128:(mt + 1) * 128],
                    rhs=x_sb[:, kt, :],
                    start=(kt == 0),
                    stop=(kt == KT - 1),
                )

            for t in range(6):               # kts 0..17
                for mt in range(NMT):
                    for kt in (3 * t, 3 * t + 1, 3 * t + 2):
                        mm(mt, kt)

            # Final k-group: finish each m-tile then immediately drain it
            # (normalize on alternating engines, store on alternating
            # rings) so only the last tile's drain is exposed.
            def drain(mt):
                o_sb = outb.tile([128, D], dt.float32, tag=f"osb{mt % 2}",
                                 name=f"osb{mt}")
                if mt % 2 == 0:
                    nc.vector.tensor_scalar_mul(
                        o_sb[:], psums[mt][:, 0:D], recip_sb[:, mt:mt + 1])
                    nc.sync.dma_start(out[mt * 128:(mt + 1) * 128, :],
                                      o_sb[:])
                else:
                    nc.scalar.activation(
                        o_sb[:], psums[mt][:, 0:D],
                        mybir.ActivationFunctionType.Copy,
                        scale=recip_sb[:, mt:mt + 1])
                    nc.scalar.dma_start(out[mt * 128:(mt + 1) * 128, :],
                                        o_sb[:])

            for mt in range(NMT):
                for kt in (18, 19, 20):
                    mm(mt, kt)
                if mt == NMT - 1:
                    # tiny columns 640/641: contract only k-tile 20
                    # (gathered rows + leftover senders 2560/2561)
                    nc.tensor.matmul(
                        pst[:, :],
                        lhsT=a_sb[:, KT - 1, WMAIN:W],
                        rhs=x_sb[:, KT - 1, :],
                        start=True,
                        stop=True,
                    )
                drain(mt)

            ot = outb.tile([2, D], dt.float32, tag="osbt", name="osb_t")
            nc.scalar.activation(ot[:], pst[:, 0:D],
                                 mybir.ActivationFunctionType.Copy,
                                 scale=recip_sb[0:2, 5:6])
            nc.scalar.dma_start(out[WMAIN:W, :], ot[:])

    nc.finalize()
    return nc


def _get_nc():
    global _NC_CACHE
    if _NC_CACHE is None:
        _NC_CACHE = _build_bass()
    return _NC_CACHE


def _pm(a):
    """[GP, F] row-major -> [SC, 128, PL, F] 3-plane partition-minor."""
    return np.ascontiguousarray(
        a.reshape(SC, PL, 128, a.shape[1]).transpose(0, 2, 1, 3))


def _host_shard(grid_node_features, edge_index):
    """Dedup edges and lay them out as per-chunk dense adjacency + padded
    bf16 x + reciprocal degrees. Returns per-core input maps."""
    x = np.asarray(grid_node_features)
    e = np.asarray(edge_index)
    g = e[:, 0].astype(np.int64)
    m = e[:, 1].astype(np.int64)
    key = np.unique(g * M + m)           # set semantics
    g = key // M
    m = key % M
    deg = np.bincount(m, minlength=M).astype(np.float64)
    rec_full = (1.0 / np.maximum(deg, 1.0)).astype(np.float32)

    ONE16 = np.uint16(0x3F80)            # bf16 1.0

    adjs = []
    recs = []
    glists = []
    for q in range(NQ):
        lo = q * W
        sel = (m >= lo) & (m < lo + W)
        gq = g[sel]
        mq = m[sel] - lo
        av = np.zeros((GP, W), np.uint16)
        # main columns: all senders (incl. 2560/2561 -> k-tile 20 rows)
        main = mq < WMAIN
        av[gq[main], mq[main]] = ONE16
        # tiny columns 640/641: senders >= 2560 sit in k-tile 20 already;
        # senders < 2560 are gathered into pad rows.
        glist = []
        for r in (WMAIN, WMAIN + 1):
            if lo + r >= M:
                continue
            snd = np.sort(gq[mq == r])
            av[snd[snd >= 2560], r] = ONE16
            for s in snd[snd < 2560]:
                av[PAD0 + len(glist), r] = ONE16
                glist.append(s)
        if len(glist) > NPAD:
            raise ValueError(f"gather overflow: {len(glist)} > {NPAD}")
        adjs.append(_pm(av).view(ml_dtypes.bfloat16))
        glists.append(np.asarray(glist, np.int64))

        rv = np.zeros((128, 6), np.float32)
        for mt in range(NMT):
            c0 = lo + mt * 128
            n = min(128, max(M - c0, 0))
            if n > 0:
                rv[:n, mt] = rec_full[c0:c0 + n]
        for j in range(2):
            if lo + WMAIN + j < M:
                rv[j, 5] = rec_full[lo + WMAIN + j]
        recs.append(rv)

    in_maps = [None] * N_CORES
    for b in range(B):
        xb = x[b, :SEN, :].astype(ml_dtypes.bfloat16)
        for q in range(NQ):
            xp = np.zeros((GP, D), ml_dtypes.bfloat16)
            xp[:SEN] = xb
            gl = glists[q]
            if gl.size:
                xp[PAD0:PAD0 + gl.size] = xb[gl]
            in_maps[b * NQ + q] = {"xh": _pm(xp), "adj": adjs[q],
                                   "recip": recs[q]}
    return in_maps


def kernel(grid_node_features, edge_index):
    from concourse.bass_utils import run_bass_kernel_spmd

    nc = _get_nc()
    in_maps = _host_shard(grid_node_features, edge_index)
    res = run_bass_kernel_spmd(nc, in_maps, core_ids=list(range(N_CORES)))

    out = np.empty((B, M, D), dtype=np.float32)
    for c in range(N_CORES):
        b, q = divmod(c, NQ)
        lo = q * W
        cq = min(W, M - lo)
        out[b, lo:lo + cq, :] = res.results[c]["out"][:cq, :]
    return out
